# revision 4
# baseline (speedup 1.0000x reference)
"""Trainium2 kernel for nn_Contrast: contrastive loss over a 10000x10000
exp-cosine-similarity matrix, sharded by rows across 8 NeuronCores.

Structure:
  host (tiny, O(N*D)): 8->8->8 MLP projection of both views, row norms,
      fold 1/(n1*n2*tau) into the operands:  a = zp1/n1,  b = zp2/(n2*tau).
      Then m[i,j] = exp(a_i . b_j).
  device (O(N^2)), per core k over its 1280-row slice of a:
      for each [128 x 512] tile of a_rows @ b^T:
        PE matmul (K=8) -> PSUM
        ACT exp PSUM->SBUF with accum_out => row-sum partials (free)
        PE one-hot matmul (E_c^T @ exp_tile) accumulating column sums for
        all 200 tiles into a single [20, 512] PSUM bank
  host: subtract zero-padding contributions, add eps, diag from exact dots,
      assemble the two mean log-ratio losses.
"""

import numpy as np

import concourse.bass as bass
import concourse.bacc as bacc
import concourse.mybir as mybir
import concourse.tile as tile
from concourse.bass_utils import run_bass_kernel_spmd

TAU = 0.5
LAM = 0.5
EPS = 1e-8

N = 10000
D = 8
NCORES = 8
NPAD = 10240              # 8 cores * 1280 rows; 20 col-tiles * 512
RPC = NPAD // NCORES      # rows per core = 1280
NSTRIP = RPC // 128       # 10 strips of 128 rows
NCT = NPAD // 512         # 20 column tiles of 512
NPAD_EXTRA = NPAD - N     # 240 zero rows/cols, each contributes exp(0)=1

# "f32" is exact-path; "bf16" halves PE streaming time (matmul inputs and the
# exp tile feeding the column-sum matmul are bf16; all accumulation is fp32).
MM_DTYPE = "f32"

# column-tile groups per strip: (first col-tile, count); 3 tiles share one
# [128, 1536] PSUM tensor (3 banks) so one ACT instruction covers them.
GROUPS = [(0, 3), (3, 3), (6, 3), (9, 3), (12, 3), (15, 3), (18, 2)]


def _mybir_dt(name):
    return mybir.dt.float32 if name == "f32" else mybir.dt.bfloat16


def _np_dt(name):
    if name == "f32":
        return np.float32
    import ml_dtypes

    return ml_dtypes.bfloat16


def _build_nc(dt_name):
    dt_in = _mybir_dt(dt_name)
    f32 = mybir.dt.float32
    nc = bacc.Bacc(None)

    lhsT = nc.dram_tensor("lhsT", [D, RPC], dt_in, kind="ExternalInput")
    rhsT = nc.dram_tensor("rhsT", [D, NPAD], dt_in, kind="ExternalInput")
    eblk = nc.dram_tensor("eblk", [128, 20 * 20], dt_in, kind="ExternalInput")
    out_rowsum = nc.dram_tensor("out_rowsum", [128, NSTRIP], f32, kind="ExternalOutput")
    out_colsum = nc.dram_tensor("out_colsum", [20, 512], f32, kind="ExternalOutput")

    ngroups = len(GROUPS)
    n_onehot = NSTRIP * NCT

    with tile.TileContext(nc) as tc:
        with (
            tc.tile_pool(name="inp", bufs=1) as inp_pool,
            tc.tile_pool(name="etile", bufs=3) as etile_pool,
            tc.tile_pool(name="rowp", bufs=2) as rowp_pool,
            tc.tile_pool(name="persist", bufs=1) as persist_pool,
            tc.tile_pool(name="pmm", bufs=2, space="PSUM") as pmm_pool,
            tc.tile_pool(name="pcol", bufs=1, space="PSUM") as pcol_pool,
        ):
            lhsT_sb = inp_pool.tile([D, RPC], dt_in)
            rhsT_sb = inp_pool.tile([D, NPAD], dt_in)
            eblk_sb = inp_pool.tile([128, 20 * 20], dt_in)

            nc.sync.dma_start(out=lhsT_sb[:], in_=lhsT[:])
            nc.sync.dma_start(out=eblk_sb[:], in_=eblk[:])
            # split the big replicated operand across DMA queues
            nchunk = 8
            cw = NPAD // nchunk
            for i in range(nchunk):
                nc.sync.dma_start(
                    out=rhsT_sb[:, i * cw : (i + 1) * cw],
                    in_=rhsT[:, i * cw : (i + 1) * cw],
                )

            rowsum_sb = persist_pool.tile([128, NSTRIP], f32)
            colsum_sb = persist_pool.tile([20, 512], f32)
            colp = pcol_pool.tile([20, 512], f32)

            # software-pipeline the one-hot (column-sum) matmuls one group
            # behind the main matmuls so PE never waits on ACT
            pending = None
            onehot_idx = 0

            def flush_pending():
                nonlocal pending, onehot_idx
                if pending is None:
                    return
                et, c0, ng = pending
                for j in range(ng):
                    c = c0 + j
                    nc.tensor.matmul(
                        colp[:, :],
                        eblk_sb[:, c * 20 : (c + 1) * 20],
                        et[:, j * 512 : (j + 1) * 512],
                        start=(onehot_idx == 0),
                        stop=(onehot_idx == n_onehot - 1),
                        skip_group_check=True,
                    )
                    onehot_idx += 1
                pending = None

            for r in range(NSTRIP):
                rowp = rowp_pool.tile([128, ngroups], f32)
                for gi, (c0, ng) in enumerate(GROUPS):
                    pa = pmm_pool.tile([128, 1536], f32)
                    for j in range(ng):
                        c = c0 + j
                        nc.tensor.matmul(
                            pa[:, j * 512 : (j + 1) * 512],
                            lhsT_sb[:, r * 128 : (r + 1) * 128],
                            rhsT_sb[:, c * 512 : (c + 1) * 512],
                            start=True,
                            stop=True,
                        )
                    et = etile_pool.tile([128, 1536], dt_in)
                    nc.scalar.activation(
                        et[:, : ng * 512],
                        pa[:, : ng * 512],
                        mybir.ActivationFunctionType.Exp,
                        accum_out=rowp[:, gi : gi + 1],
                    )
                    flush_pending()
                    pending = (et, c0, ng)
                nc.vector.reduce_sum(
                    out=rowsum_sb[:, r : r + 1],
                    in_=rowp[:, :],
                    axis=mybir.AxisListType.X,
                )
            flush_pending()

            nc.vector.tensor_copy(out=colsum_sb[:], in_=colp[:])
            nc.sync.dma_start(out=out_rowsum[:], in_=rowsum_sb[:])
            nc.sync.dma_start(out=out_colsum[:], in_=colsum_sb[:])

    nc.compile()
    return nc


_NC_CACHE = {}


def _get_nc(dt_name):
    if dt_name not in _NC_CACHE:
        _NC_CACHE[dt_name] = _build_nc(dt_name)
    return _NC_CACHE[dt_name]


def _proj_np(z, W1, b1, W2, b2):
    h = z @ W1.T + b1
    h = np.where(h > 0, h, np.expm1(h)).astype(np.float32)
    return (h @ W2.T + b2).astype(np.float32)


def _prepare_operands(z_mp, z_sc, W1, b1, W2, b2):
    zp1 = _proj_np(z_mp.astype(np.float32), W1, b1, W2, b2)
    zp2 = _proj_np(z_sc.astype(np.float32), W1, b1, W2, b2)
    n1 = np.sqrt(np.sum(zp1 * zp1, axis=1, keepdims=True)).astype(np.float32)
    n2 = np.sqrt(np.sum(zp2 * zp2, axis=1, keepdims=True)).astype(np.float32)
    a = (zp1 / n1).astype(np.float32)
    b = (zp2 / (n2 * np.float32(TAU))).astype(np.float32)
    dots = np.sum(a * b, axis=1).astype(np.float32)  # diag logits (exact path)
    return a, b, dots


def kernel(z_mp, z_sc, W1, b1, W2, b2):
    a, b, dots = _prepare_operands(z_mp, z_sc, W1, b1, W2, b2)

    np_dt = _np_dt(MM_DTYPE)
    a_pad = np.zeros((NPAD, D), np.float32)
    a_pad[:N] = a
    b_pad = np.zeros((NPAD, D), np.float32)
    b_pad[:N] = b
    aT = np.ascontiguousarray(a_pad.T).astype(np_dt)
    bT = np.ascontiguousarray(b_pad.T).astype(np_dt)
    E = np.tile(np.eye(20, dtype=np_dt)[None], (128, 1, 1)).reshape(128, 400)
    E = np.ascontiguousarray(E)

    nc = _get_nc(MM_DTYPE)
    in_maps = [
        {
            "lhsT": np.ascontiguousarray(aT[:, k * RPC : (k + 1) * RPC]),
            "rhsT": bT,
            "eblk": E,
        }
        for k in range(NCORES)
    ]
    res = run_bass_kernel_spmd(nc, in_maps, list(range(NCORES))).results

    rowsum_full = np.concatenate(
        [np.asarray(res[k]["out_rowsum"]).T.reshape(-1) for k in range(NCORES)]
    )
    colsum_full = np.sum(
        [np.asarray(res[k]["out_colsum"]).reshape(-1) for k in range(NCORES)], axis=0
    )

    row_sum = rowsum_full[:N].astype(np.float64) - NPAD_EXTRA + EPS
    col_sum = colsum_full[:N].astype(np.float64) - NPAD_EXTRA + EPS
    diag = np.exp(dots.astype(np.float64))
    lori_mp = -np.mean(np.log(diag / row_sum))
    lori_sc = -np.mean(np.log(diag / col_sum))
    return np.float32(LAM * lori_mp + (1.0 - LAM) * lori_sc)


# revision 28
# speedup vs baseline: 654.9563x; 654.9563x over previous
"""Trainium2 kernel for nn_Contrast: contrastive loss over a 10000x10000
exp-cosine-similarity matrix, sharded by rows across 8 NeuronCores.

Structure:
  host (tiny, O(N*D)): 8->8->8 MLP projection of both views, row norms,
      fold 1/(n1*n2*tau) into the operands:  a = zp1/n1,  b = zp2/(n2*tau).
      Then m[i,j] = exp(a_i . b_j).
  device (O(N^2)), per core k over its 1280-row slice of a:
      for each [128 x <=512] tile of a_rows @ b^T:
        PE matmul (K=8) -> PSUM (3 tiles share a 3-bank PSUM tensor)
        ACT exp PSUM->SBUF with accum_out => row-sum partials (free)
        PE one-hot matmul (E_c^T @ exp_tile) accumulating column sums for
        all tiles into a single [20, 512] PSUM bank
  host: subtract zero-padding contributions, add eps, diag from exact dots,
      assemble the two mean log-ratio losses.
"""

import numpy as np

import concourse.bass as bass
import concourse.bacc as bacc
import concourse.mybir as mybir
import concourse.tile as tile
from concourse.bass_utils import run_bass_kernel_spmd

TAU = 0.5
LAM = 0.5
EPS = 1e-8

N = 10000
D = 8
NCORES = 8
RPAD = 10240              # lhs rows padded: 8 cores * 1280
RPC = RPAD // NCORES      # rows per core = 1280
NSTRIP = RPC // 128       # 10 strips of 128 rows
ROW_PAD = RPAD - N        # 240 zero lhs rows -> contribute exp(0)=1 per column

# column tiles cover exactly N columns: 19 x 512 + 272
COL_TILES = [(c * 512, min(512, N - c * 512)) for c in range((N + 511) // 512)]
NCT = len(COL_TILES)      # 20
# tiles per strip are grouped so each group's matmuls share one PSUM tensor
# and one ACT(exp) instruction. PSUM budget is 8 banks: 2 x 3-bank tensors
# (double-buffered) + 1 bank for the column-sum accumulator. The short
# (2-tile, 784-wide) group goes first in each strip: a short ACT instruction
# in the middle of a strip stalls the pipeline less there.
GROUPS = [COL_TILES[18:20]] + [COL_TILES[i : i + 3] for i in range(0, 18, 3)]

# "f32" is the exact-but-slow path (PE runs fp32 at 4 cycles/row).
# "f32r" streams fp32 bits through the PE at full rate with relaxed rounding;
# "bf16" is the same speed with coarser rounding and no staging copies.
# All accumulation stays fp32 and the scalar loss averages the per-element
# rounding noise away (measured loss rel err: bf16 0.0, f32r 1e-7, f32 1e-7).
MM_DTYPE = "bf16"


def _mybir_dt(name):
    return {
        "f32": mybir.dt.float32,
        "f32r": mybir.dt.float32r,
        "bf16": mybir.dt.bfloat16,
    }[name]


def _np_dt(name):
    if name in ("f32", "f32r"):
        return np.float32
    import ml_dtypes

    return ml_dtypes.bfloat16


def _build_nc(dt_name):
    dt_in = _mybir_dt(dt_name)
    f32 = mybir.dt.float32
    nc = bacc.Bacc(None)

    dram_dt = mybir.dt.bfloat16 if dt_name == "bf16" else mybir.dt.float32
    lhsT = nc.dram_tensor("lhsT", [D, RPC], dram_dt, kind="ExternalInput")
    rhsT = nc.dram_tensor("rhsT", [D, N], dram_dt, kind="ExternalInput")
    eblk = nc.dram_tensor("eblk", [128, NCT * 20], dram_dt, kind="ExternalInput")
    out_rowsum = nc.dram_tensor("out_rowsum", [128, NSTRIP], f32, kind="ExternalOutput")
    out_colsum = nc.dram_tensor("out_colsum", [20, 512], f32, kind="ExternalOutput")

    ngroups = len(GROUPS)
    n_onehot = NSTRIP * NCT

    with tile.TileContext(nc) as tc:
        with (
            tc.tile_pool(name="inp", bufs=1) as inp_pool,
            tc.tile_pool(name="etile", bufs=3) as etile_pool,
            tc.tile_pool(name="rowp", bufs=2) as rowp_pool,
            tc.tile_pool(name="persist", bufs=1) as persist_pool,
            tc.tile_pool(name="pmm", bufs=2, space="PSUM") as pmm_pool,
            tc.tile_pool(name="pcol", bufs=1, space="PSUM") as pcol_pool,
        ):
            lhsT_sb = inp_pool.tile([D, RPC], dt_in)
            rhsT_sb = inp_pool.tile([D, N], dt_in)
            eblk_sb = inp_pool.tile([128, NCT * 20], dt_in)

            if dt_name == "f32r":
                # f32r operands need a rounding producer; sync-DMA into f32
                # staging, then idle-DVE copies do the cast. Chunked so the
                # first matmuls start as soon as their span is staged; eblk is
                # only needed by the first one-hot matmul (~8us in), so it
                # loads after the first two rhs chunks.
                lhsT_st = inp_pool.tile([D, RPC], f32)
                rhsT_st = inp_pool.tile([D, N], f32)
                eblk_st = inp_pool.tile([128, NCT * 20], f32)

                # each dma_start costs ~650ns of serial sequencer issue, so
                # the pieces feeding the first matmuls go first and the bulk
                # follows in a few large DMAs. DVE cast copies are chunked in
                # group-consumption order so compute starts as data rounds.
                spans = []
                for grp in GROUPS:
                    g0 = grp[0][0]
                    spans.append((g0, g0 + sum(w for _, w in grp)))
                rest = sorted(spans[2:])  # contiguous ascending tail spans
                nc.sync.dma_start(out=lhsT_st[:, 0:128], in_=lhsT[:, 0:128])
                nc.sync.dma_start(
                    out=rhsT_st[:, spans[0][0] : spans[0][1]],
                    in_=rhsT[:, spans[0][0] : spans[0][1]],
                )
                nc.sync.dma_start(
                    out=rhsT_st[:, spans[1][0] : spans[1][1]],
                    in_=rhsT[:, spans[1][0] : spans[1][1]],
                )
                nc.sync.dma_start(out=lhsT_st[:, 128:RPC], in_=lhsT[:, 128:RPC])
                nc.sync.dma_start(
                    out=rhsT_st[:, rest[0][0] : rest[2][1]],
                    in_=rhsT[:, rest[0][0] : rest[2][1]],
                )
                nc.sync.dma_start(out=eblk_st[:], in_=eblk[:])
                nc.sync.dma_start(
                    out=rhsT_st[:, rest[3][0] : rest[-1][1]],
                    in_=rhsT[:, rest[3][0] : rest[-1][1]],
                )

                def _cast(dst, st, lo, hi):
                    nc.vector.tensor_copy(out=dst[:, lo:hi], in_=st[:, lo:hi])

                _cast(lhsT_sb, lhsT_st, 0, 128)
                _cast(rhsT_sb, rhsT_st, *spans[0])
                _cast(lhsT_sb, lhsT_st, 128, RPC)
                _cast(rhsT_sb, rhsT_st, *spans[1])
                _cast(eblk_sb, eblk_st, 0, NCT * 20)
                for sp in spans[2:]:
                    _cast(rhsT_sb, rhsT_st, *sp)
            else:
                nc.sync.dma_start(out=lhsT_sb[:], in_=lhsT[:])
                for grp in GROUPS[:2]:
                    g0 = grp[0][0]
                    gw = sum(w for _, w in grp)
                    nc.sync.dma_start(
                        out=rhsT_sb[:, g0 : g0 + gw], in_=rhsT[:, g0 : g0 + gw]
                    )
                nc.sync.dma_start(out=eblk_sb[:], in_=eblk[:])
                for grp in GROUPS[2:]:
                    g0 = grp[0][0]
                    gw = sum(w for _, w in grp)
                    nc.sync.dma_start(
                        out=rhsT_sb[:, g0 : g0 + gw], in_=rhsT[:, g0 : g0 + gw]
                    )

            rowsum_sb = persist_pool.tile([128, NSTRIP], f32)
            colsum_sb = persist_pool.tile([20, 512], f32)
            colp = pcol_pool.tile([20, 512], f32)

            # software-pipeline the one-hot (column-sum) matmuls one group
            # behind the main matmuls so PE never waits on ACT
            pending = None
            onehot_idx = 0

            def flush_pending():
                nonlocal pending, onehot_idx
                if pending is None:
                    return
                et, grp = pending
                off = 0
                for c0, w in grp:
                    c = c0 // 512  # global column-tile index = colp row
                    nc.tensor.matmul(
                        colp[:, 0:w],
                        eblk_sb[:, c * 20 : (c + 1) * 20],
                        et[:, off : off + w],
                        start=(onehot_idx == 0),
                        stop=(onehot_idx == n_onehot - 1),
                        skip_group_check=True,
                    )
                    off += w
                    onehot_idx += 1
                pending = None

            for r in range(NSTRIP):
                rowp = rowp_pool.tile([128, ngroups], f32)
                for gi, grp in enumerate(GROUPS):
                    gw = sum(w for _, w in grp)
                    pa = pmm_pool.tile([128, 1536], f32, name=f"pa_{r}_{gi}", tag="pa")
                    off = 0
                    for c0, w in grp:
                        nc.tensor.matmul(
                            pa[:, off : off + w],
                            lhsT_sb[:, r * 128 : (r + 1) * 128],
                            rhsT_sb[:, c0 : c0 + w],
                            start=True,
                            stop=True,
                        )
                        off += w
                    et = etile_pool.tile([128, 1536], dt_in)
                    nc.scalar.activation(
                        et[:, :gw],
                        pa[:, :gw],
                        mybir.ActivationFunctionType.Exp,
                        accum_out=rowp[:, gi : gi + 1],
                    )
                    flush_pending()
                    pending = (et, grp)
                nc.vector.reduce_sum(
                    out=rowsum_sb[:, r : r + 1],
                    in_=rowp[:, :],
                    axis=mybir.AxisListType.X,
                )
            flush_pending()

            nc.vector.tensor_copy(out=colsum_sb[:], in_=colp[:])
            nc.sync.dma_start(out=out_rowsum[:], in_=rowsum_sb[:])
            nc.sync.dma_start(out=out_colsum[:], in_=colsum_sb[:])

    nc.compile()
    return nc


_NC_CACHE = {}


def _get_nc(dt_name):
    if dt_name not in _NC_CACHE:
        _NC_CACHE[dt_name] = _build_nc(dt_name)
    return _NC_CACHE[dt_name]


def _proj_np(z, W1, b1, W2, b2):
    h = z @ W1.T + b1
    h = np.where(h > 0, h, np.expm1(h)).astype(np.float32)
    return (h @ W2.T + b2).astype(np.float32)


def _prepare_operands(z_mp, z_sc, W1, b1, W2, b2):
    zp1 = _proj_np(z_mp.astype(np.float32), W1, b1, W2, b2)
    zp2 = _proj_np(z_sc.astype(np.float32), W1, b1, W2, b2)
    n1 = np.sqrt(np.sum(zp1 * zp1, axis=1, keepdims=True)).astype(np.float32)
    n2 = np.sqrt(np.sum(zp2 * zp2, axis=1, keepdims=True)).astype(np.float32)
    a = (zp1 / n1).astype(np.float32)
    b = (zp2 / (n2 * np.float32(TAU))).astype(np.float32)
    dots = np.sum(a * b, axis=1).astype(np.float32)  # diag logits (exact path)
    return a, b, dots


def _make_in_maps(a, b):
    np_dt = _np_dt(MM_DTYPE)
    a_pad = np.zeros((RPAD, D), np.float32)
    a_pad[:N] = a
    aT = np.ascontiguousarray(a_pad.T).astype(np_dt)
    bT = np.ascontiguousarray(b.T).astype(np_dt)
    E = np.ascontiguousarray(
        np.tile(np.eye(20, dtype=np_dt)[None], (128, 1, 1)).reshape(128, NCT * 20)
    )
    return [
        {
            "lhsT": np.ascontiguousarray(aT[:, k * RPC : (k + 1) * RPC]),
            "rhsT": bT,
            "eblk": E,
        }
        for k in range(NCORES)
    ]


def _finalize(res, dots):
    rowsum_full = np.concatenate(
        [np.asarray(res[k]["out_rowsum"]).T.reshape(-1) for k in range(NCORES)]
    )
    colsum_full = np.sum(
        [np.asarray(res[k]["out_colsum"]).reshape(-1) for k in range(NCORES)], axis=0
    )
    row_sum = rowsum_full[:N].astype(np.float64) + EPS
    col_sum = colsum_full[:N].astype(np.float64) - ROW_PAD + EPS
    diag = np.exp(dots.astype(np.float64))
    lori_mp = -np.mean(np.log(diag / row_sum))
    lori_sc = -np.mean(np.log(diag / col_sum))
    return np.float32(LAM * lori_mp + (1.0 - LAM) * lori_sc)


def kernel(z_mp, z_sc, W1, b1, W2, b2):
    a, b, dots = _prepare_operands(z_mp, z_sc, W1, b1, W2, b2)
    in_maps = _make_in_maps(a, b)
    nc = _get_nc(MM_DTYPE)
    res = run_bass_kernel_spmd(nc, in_maps, list(range(NCORES))).results
    return _finalize(res, dots)
